# revision 3
# baseline (speedup 1.0000x reference)
"""Trainium2 Bass kernel for nn_CONV_tiny_add_partial_558345748883 (v5).

Network: 3x [conv5x5(pad2) -> BN -> avgpool2 -> clip01] -> conv4x4(valid) -> BN1d
Input x_in [1024, 3, 32, 32] f32; output [1024, 10] f32.

Key facts driving the design (measured on HW):
- Tensor rounds of 8 concurrent 32-wide tile_position matmuls sustain
  ~273ns per 512-col round; the pace collapses ~1.3-1.7x when the rhs
  inner walk is stride-2 AND its 16B line phase straddles an extra line.
  Contiguous inner runs are immune at any offset, so all activation
  tensors store column-parity planes: the conv dx-taps become
  (plane, offset) with stride-1 inner reads.
- L1 contraction K=36 (column-parity baked host-side into fp8 x_l1),
  3 dx passes, 2 row strips x 4 col tiles, fp8 input+weights.
- PE HAM clock gate: warm-up dummy matmuls at t=0 and small dummy bursts
  at phase barriers keep the PE at 2.4GHz.
- GpSimd is never used for compute or DMA (both are slow paths).
- Emission: L1 sets 0..7 (DMA-paced), L2w0, L2w1, L3(both waves), L4,
  with PSUM rings tagged so no phase waits on the previous one's evac.
"""
import os
import sys
import numpy as np

for _p in ("/opt/trn_rl_repo", "/root/.axon_site/_ro/trn_rl_repo"):
    if os.path.isdir(_p) and _p not in sys.path:
        sys.path.append(_p)

import concourse.bass as bass
import concourse.bacc as bacc
import concourse.mybir as mybir
from concourse.tile import TileContext

EPS = 1e-5
N_CORES = 8
DT = mybir.dt.float16
DT8 = mybir.dt.float8e4
F32 = mybir.dt.float32
AF = mybir.ActivationFunctionType
NP8 = mybir.dt.np(DT8)

Q = 16                  # l2t slots per lane per wave
NW = 2
S = 128
NSET = 8
H2 = Q // 2
NQ = NW * Q

# l2 plane window: [fb 2][y 20][x' 10] ; l3 plane window: [fb 2][y 12][x' 6]
L2SLOT = 2 * 20 * 10    # 400
L3SLOT = 2 * 12 * 8     # 192 (x-pad 8: 16B-aligned rows)

TAPS = [(e, f) for e in range(6) for f in range(6)]


def _fold_w(w, g, b, m, v):
    inv = g / np.sqrt(v + EPS)
    Wp = np.zeros((w.shape[0], w.shape[1], 6, 6), np.float32)
    for r in (0, 1):
        for s_ in (0, 1):
            Wp[:, :, r:r + 5, s_:s_ + 5] += w
    Wp *= 0.25 * inv[:, None, None, None]
    beta = (b - m * inv).astype(np.float32)
    return Wp.astype(np.float32), beta


def _lane_rep(a, groups=4):
    out = np.zeros((128, a.shape[1]), np.float32)
    for j in range(groups):
        out[32 * j:32 * j + a.shape[0]] = a
    return out


def host_prep_weights(inputs):
    W1, beta1 = _fold_w(inputs['w1'], inputs['g1'], inputs['b1'], inputs['m1'], inputs['v1'])
    W2, beta2 = _fold_w(inputs['w2'], inputs['g2'], inputs['b2'], inputs['m2'], inputs['v2'])
    W3, beta3 = _fold_w(inputs['w3'], inputs['g3'], inputs['b3'], inputs['m3'], inputs['v3'])
    inv4 = inputs['g4'] / np.sqrt(inputs['v4'] + EPS)
    beta4 = (inputs['b4'] - inputs['m4'] * inv4).astype(np.float32)
    W4 = (inputs['w4'] * inv4[:, None, None, None]).astype(np.float32)

    d = {}
    t1 = W1.reshape(32, 3, 6, 3, 2)            # co, ci, dy, f2, fb
    w36 = t1.transpose(4, 2, 1, 3, 0).reshape(36, 96)
    wl1 = np.zeros((128, 96), np.float32)
    wl1[0:36] = w36
    wl1[64:100] = w36
    d['wl1'] = wl1.astype(NP8)
    d['wl2'] = _lane_rep(W2.transpose(1, 2, 3, 0).reshape(32, 36 * 32)).astype(np.float16)
    d['wl3'] = _lane_rep(W3.transpose(1, 2, 3, 0).reshape(32, 36 * 64)).astype(np.float16)
    wl4 = W4.transpose(1, 2, 3, 0).reshape(64, 16 * 10)
    out4 = np.zeros((128, 160), np.float32)
    out4[0:64] = wl4
    out4[64:128] = wl4
    d['wl4'] = out4.astype(np.float16)

    bt = np.zeros((128, 4), np.float32)
    bt[:, 0] = np.tile(beta1, 4)
    bt[:, 1] = np.tile(beta2, 4)
    bt[:, 2] = np.tile(beta3, 2)
    bt[0:10, 3] = beta4
    d['betas'] = bt
    return d


def host_prep_x(x_core):
    """[128, 3, 32, 32] -> x_l1 [2, 36, 64, 288] column-parity im2row (fp8)."""
    Sc = x_core.shape[0]
    xp = np.zeros((Sc, 3, 36, 36), np.float32)
    xp[:, :, 2:34, 2:34] = x_core
    a = np.stack([xp[:, :, dy:dy + 32:2, :] for dy in range(6)], axis=1)
    b = np.stack([a[..., fb::2] for fb in range(2)], axis=1)
    c = b.reshape(Sc, 36, 288)
    x_l1 = c.reshape(2, 64, 36, 288).transpose(0, 2, 1, 3)
    return np.ascontiguousarray(x_l1).astype(NP8)


def build_program():
    nc = bacc.Bacc(target_bir_lowering=False)

    x_l1 = nc.dram_tensor("x_l1", [2, 36, 64, 288], DT8, kind="ExternalInput")
    wl1 = nc.dram_tensor("wl1", [128, 96], DT8, kind="ExternalInput")
    wl2 = nc.dram_tensor("wl2", [128, 1152], DT, kind="ExternalInput")
    wl3 = nc.dram_tensor("wl3", [128, 2304], DT, kind="ExternalInput")
    wl4 = nc.dram_tensor("wl4", [128, 160], DT, kind="ExternalInput")
    betas = nc.dram_tensor("betas", [128, 4], F32, kind="ExternalInput")
    y = nc.dram_tensor("y", [10, 128], F32, kind="ExternalOutput")

    with TileContext(nc) as tc:
        with tc.tile_pool(name="consts", bufs=1) as cpool, \
             tc.tile_pool(name="ps", bufs=4, space="PSUM") as pspool:
            wdum = cpool.tile([128, 32], DT, name="wdum")
            rdum = cpool.tile([128, 512], DT, name="rdum")
            nc.gpsimd.memset(wdum[:, :], 0.03)
            nc.gpsimd.memset(rdum[:, :], 0.02)

            wl1_t = cpool.tile([128, 96], DT8, name="wl1_t")
            wl2_t = cpool.tile([128, 1152], DT, name="wl2_t")
            wl3_t = cpool.tile([128, 2304], DT, name="wl3_t")
            wl4_t = cpool.tile([128, 160], DT, name="wl4_t")
            betas_t = cpool.tile([128, 4], F32, name="betas_t")
            nc.sync.dma_start(wl1_t[:, :], wl1.ap())
            nc.sync.dma_start(betas_t[:, :], betas.ap())

            # activation tensors (column-parity plane layouts)
            l2in = [cpool.tile([128, Q * L2SLOT], DT, name=f"l2in{i}") for i in range(2)]
            l3in = cpool.tile([128, 2 * Q * L3SLOT], DT, name="l3in")
            stag = cpool.tile([128, 2 * NQ * 16], DT, name="stag")
            out_sb = cpool.tile([128, 128], F32, name="out_sb")

            # input chunks: one tile per 2 sets, first pair split per set
            chunks = [cpool.tile([128, 16 * 288], DT8, name=f"ch{i}") for i in range(4)]
            _issued = set()

            def issue_pair(i, split=False):
                if i in _issued:
                    return
                _issued.add(i)
                chd = chunks[i].rearrange("p (j v) -> p j v", v=288)
                for L in range(2):
                    eng = [nc.sync, nc.scalar][L]
                    if split:
                        for h in range(2):
                            eng.dma_start(
                                chd[64 * L:64 * L + 36, 8 * h:8 * h + 8, :],
                                x_l1.ap()[L, :, 16 * i + 8 * h:16 * i + 8 * h + 8, :])
                    else:
                        eng.dma_start(chd[64 * L:64 * L + 36, 0:16, :],
                                      x_l1.ap()[L, :, 16 * i:16 * i + 16, :])

            issue_pair(0, split=True)
            issue_pair(1)

            # ---- zero the window tensors once (borders stay zero; the
            # evacs overwrite interiors and the clips keep zeros zero).
            # tensor_scalar_mul runs in 4x DVE mode; MEMSET only 1x. ----
            for t_ in l2in:
                nc.vector.tensor_scalar_mul(t_[:, :], t_[:, :], 0.0)
            nc.vector.tensor_scalar_mul(l3in[:, :], l3in[:, :], 0.0)

            psd = [pspool.tile([128, 512], F32, name=f"psd{i}", tag="l1") for i in range(2)]
            for i in range(12):
                nc.tensor.matmul(psd[i % 2][0:32, 0:512], wdum[0:128, 0:32],
                                 rdum[0:128, 0:512], start=True, stop=True,
                                 skip_group_check=True, tile_position=(0, 0))

            issue_pair(2)
            issue_pair(3)
            nc.scalar.dma_start(wl2_t[:, :], wl2.ap())
            nc.scalar.dma_start(wl3_t[:, :], wl3.ap())
            nc.scalar.dma_start(wl4_t[:, :], wl4.ap())

            def evac(engine_is_act, dst, src_ap, bias_ap):
                if engine_is_act:
                    nc.scalar.activation(dst, src_ap, AF.Relu,
                                         bias=bias_ap, scale=1.0)
                else:
                    nc.vector.tensor_scalar(
                        dst, src_ap, bias_ap, 0.0,
                        mybir.AluOpType.add, mybir.AluOpType.max)

            def dummy_burst(nm, n):
                pd = pspool.tile([128, 512], F32, name=nm, tag="l1")
                for _ in range(n):
                    nc.tensor.matmul(pd[0:32, 0:512], wdum[0:128, 0:32],
                                     rdum[0:128, 0:512], start=True, stop=True,
                                     skip_group_check=True, tile_position=(0, 0))

            # ================= L1 (one 16-sample set) =================
            l2vv = [t_.rearrange("p (s fb y x) -> p s fb y x", s=Q, fb=2, y=20)
                    for t_ in l2in]

            def emit_L1_set(t):
                w, u = t // 4, t % 4
                chv = chunks[t // 2].rearrange("p (j y x) -> p j y x", j=16, y=16)
                j0 = 8 * (t % 2)
                ps = [pspool.tile([128, 512], F32, name=f"ps1_{t}_{L}", tag="l1")
                      for L in range(2)]
                for f2 in range(3):
                    for L in range(2):
                        lhsT = wl1_t[64 * L:64 * L + 36, 32 * f2:32 * f2 + 32]
                        for C in range(4):
                            rhs = chv[64 * L:64 * L + 36, j0 + 2 * C:j0 + 2 * C + 2,
                                      0:16, f2:f2 + 16]
                            nc.tensor.matmul(
                                ps[L][32 * C:32 * C + 32, 0:512], lhsT, rhs,
                                start=(f2 == 0), stop=(f2 == 2),
                                skip_group_check=True,
                                tile_position=(64 * L, 32 * C))
                # evac into parity planes: out col 2+X -> plane X%2, x' = 1 + X//2
                vv = l2vv[w % 2]
                for L in range(2):
                    m0 = 4 * u + 2 * L
                    srcv = ps[L][0:128, :].rearrange(
                        "p (k yy xx) -> p k yy xx", k=2, yy=16)
                    for fb in range(2):
                        src_ap = srcv[:, :, :, fb:16:2]
                        dst = vv[0:128, m0:m0 + 2, fb, 2:18, 1:9]
                        eng_act = (L == 0) or (t % 2 == 1 and fb == 0)
                        evac(eng_act, dst, src_ap, betas_t[0:128, 0:1])
                if t % 2 == 1:
                    base = 4 * (u - 1) * L2SLOT
                    nc.vector.tensor_scalar_min(
                        l2in[w % 2][:, base:base + 8 * L2SLOT],
                        l2in[w % 2][:, base:base + 8 * L2SLOT], 1.0)

            # ================= L2 (one wave) =================
            l3vv = l3in.rearrange("p (s fb y x) -> p s fb y x", s=2 * Q, fb=2, y=12)

            def emit_L2(w, tag):
                l2v = l2vv[w % 2]
                pl2 = [pspool.tile([128, 512], F32, name=f"ps2_{w}_{r}", tag=tag)
                       for r in range(4)]
                for t, (e, f) in enumerate(TAPS):
                    fb, f2 = f % 2, f // 2
                    for r2 in range(4):
                        lhsT = wl2_t[32 * r2:32 * r2 + 32, 32 * t:32 * t + 32]
                        for c2 in range(2):
                            rhs = l2v[32 * r2:32 * r2 + 32,
                                      H2 * c2:H2 * (c2 + 1), fb,
                                      e:e + 15:2, f2:f2 + 8]
                            nc.tensor.matmul(
                                pl2[r2][32 * c2:32 * c2 + 32, 0:H2 * 64], lhsT, rhs,
                                start=(t == 0), stop=(t == 35),
                                skip_group_check=True,
                                tile_position=(32 * r2, 32 * c2))
                for r2 in range(4):
                    gb = 64 * (r2 % 2)
                    slot0 = w * Q + H2 * (r2 // 2)
                    srcv = pl2[r2][0:64, 0:H2 * 64].rearrange(
                        "p (k yy xx) -> p k yy xx", k=H2, yy=8)
                    for fb in range(2):
                        src_ap = srcv[:, :, :, fb:8:2]
                        dst = l3vv[gb:gb + 64, slot0:slot0 + H2, fb, 2:10, 1:5]
                        evac(r2 < 2, dst, src_ap, betas_t[gb:gb + 64, 1:2])
                nc.vector.tensor_scalar_min(
                    l3in[:, w * Q * L3SLOT:(w + 1) * Q * L3SLOT],
                    l3in[:, w * Q * L3SLOT:(w + 1) * Q * L3SLOT], 1.0)

            # ================= L3 (both waves) =================
            sv = stag.rearrange("p (g n t) -> p g n t", g=2, n=NQ)

            def emit_L3(tag):
                pl3 = [pspool.tile([128, 512], F32, name=f"ps3_{r}", tag=tag)
                       for r in range(4)]
                for t, (e, f) in enumerate(TAPS):
                    fb, f2 = f % 2, f // 2
                    for r3 in range(4):
                        rhs = l3vv[32 * r3:32 * r3 + 32, :, fb, e:e + 7:2, f2:f2 + 4]
                        for h in range(2):
                            lhsT = wl3_t[32 * r3:32 * r3 + 32,
                                         64 * t + 32 * h:64 * t + 32 * h + 32]
                            nc.tensor.matmul(
                                pl3[r3][32 * h:32 * h + 32, 0:512], lhsT, rhs,
                                start=(t == 0), stop=(t == 35),
                                skip_group_check=True,
                                tile_position=(32 * r3, 32 * h))
                for r3 in range(4):
                    c3, g = r3 // 2, r3 % 2
                    src_ap = pl3[r3][0:64, :].rearrange("p (n t) -> p n t", n=NQ)
                    dst = sv[64 * c3:64 * c3 + 64, g, :, :]
                    evac(r3 < 2, dst, src_ap, betas_t[64 * c3:64 * c3 + 64, 2:3])
                for g in range(2):
                    nc.vector.tensor_scalar_min(
                        stag[:, g * NQ * 16:(g + 1) * NQ * 16],
                        stag[:, g * NQ * 16:(g + 1) * NQ * 16], 1.0)

            # ================= L4 =================
            def emit_L4(tag):
                ps4 = [pspool.tile([128, 64], F32, name=f"ps4_{r}", tag=tag)
                       for r in range(2)]
                for t in range(16):
                    for r in range(2):
                        lhsT = wl4_t[64 * r:64 * r + 64, 10 * t:10 * t + 10]
                        rhs = sv[64 * r:64 * r + 64, :, :, t]
                        nc.tensor.matmul(
                            ps4[r][0:10, 0:64], lhsT, rhs,
                            start=(t == 0), stop=(t == 15),
                            skip_group_check=True,
                            tile_position=(64 * r, 0))
                for r in range(2):
                    nc.scalar.activation(
                        out_sb[0:10, 64 * r:64 * r + 64], ps4[r][0:10, 0:64],
                        AF.Identity, bias=betas_t[0:10, 3:4], scale=1.0)
                    nc.sync.dma_start(y.ap()[:, 64 * r:64 * r + 64],
                                      out_sb[0:10, 64 * r:64 * r + 64])

            # ---- emission order ----
            for t in range(NSET):
                emit_L1_set(t)
                if t % 2 == 1 and t < 7:
                    dummy_burst(f"dbs{t}", 2)
            dummy_burst("db0", 3)
            emit_L2(0, "l2")
            emit_L2(1, "l1")
            dummy_burst("db2", 3)
            emit_L3("l2")
            dummy_burst("db3", 5)
            emit_L4("l1")

        return nc


_NC_CACHE = None


def get_program():
    global _NC_CACHE
    if _NC_CACHE is None:
        nc = build_program()
        if not nc.is_finalized():
            nc.finalize()
        _NC_CACHE = nc
    return _NC_CACHE


def make_in_maps(inputs, n_cores=N_CORES):
    wdict = host_prep_weights(inputs)
    in_maps = []
    for c in range(n_cores):
        x_core = np.asarray(inputs['x_in'][c * S:(c + 1) * S], np.float32)
        m = {'x_l1': host_prep_x(x_core)}
        m.update(wdict)
        in_maps.append(m)
    return in_maps


def _col_to_sample(col):
    r, rem = divmod(col, 64)
    g, n = divmod(rem, 32)
    w, slot16 = divmod(n, 16)
    sh, slotrel = divmod(slot16, 8)
    r2 = 2 * sh + r
    C = r2
    m = 8 * g + slotrel
    u, mr = divmod(m, 4)
    L, k = divmod(mr, 2)
    t = 4 * w + u
    j = 8 * t + 2 * C + k
    return 64 * L + j


_COLMAP = np.array([_col_to_sample(c) for c in range(128)])


def assemble_output(results, n_cores=N_CORES):
    out = np.zeros((n_cores * S, 10), np.float32)
    for c in range(n_cores):
        yc = np.asarray(results[c]['y'])
        out[c * S + _COLMAP, :] = yc.T
    return out


def kernel(**inputs) -> np.ndarray:
    from concourse.bass_utils import run_bass_kernel_spmd
    nc = get_program()
    in_maps = make_in_maps(inputs)
    res = run_bass_kernel_spmd(nc, in_maps, list(range(N_CORES)))
    return assemble_output(res.results)


# revision 4
# speedup vs baseline: 1.0087x; 1.0087x over previous
"""Trainium2 Bass kernel for nn_CONV_tiny_add_partial_558345748883 (v5).

Network: 3x [conv5x5(pad2) -> BN -> avgpool2 -> clip01] -> conv4x4(valid) -> BN1d
Input x_in [1024, 3, 32, 32] f32; output [1024, 10] f32.

Key facts driving the design (measured on HW):
- Tensor rounds of 8 concurrent 32-wide tile_position matmuls sustain
  ~273ns per 512-col round; the pace collapses ~1.3-1.7x when the rhs
  inner walk is stride-2 AND its 16B line phase straddles an extra line.
  Contiguous inner runs are immune at any offset, so all activation
  tensors store column-parity planes: the conv dx-taps become
  (plane, offset) with stride-1 inner reads.
- L1 contraction K=36 (column-parity baked host-side into fp8 x_l1),
  3 dx passes, 2 row strips x 4 col tiles, fp8 input+weights.
- PE HAM clock gate: warm-up dummy matmuls at t=0 and small dummy bursts
  at phase barriers keep the PE at 2.4GHz.
- GpSimd is never used for compute or DMA (both are slow paths).
- Emission: L1 sets 0..7 (DMA-paced), L2w0, L2w1, L3(both waves), L4,
  with PSUM rings tagged so no phase waits on the previous one's evac.
"""
import os
import sys
import numpy as np

for _p in ("/opt/trn_rl_repo", "/root/.axon_site/_ro/trn_rl_repo"):
    if os.path.isdir(_p) and _p not in sys.path:
        sys.path.append(_p)

import concourse.bass as bass
import concourse.bacc as bacc
import concourse.mybir as mybir
from concourse.tile import TileContext

EPS = 1e-5
N_CORES = 8
DT = mybir.dt.float16
DT8 = mybir.dt.float8e4
F32 = mybir.dt.float32
AF = mybir.ActivationFunctionType
NP8 = mybir.dt.np(DT8)

Q = 16                  # l2t slots per lane per wave
NW = 2
S = 128
NSET = 8
H2 = Q // 2
NQ = NW * Q

# l2 plane window: [fb 2][y 20][x' 10] ; l3 plane window: [fb 2][y 12][x' 6]
L2SLOT = 2 * 20 * 10    # 400
L3SLOT = 2 * 12 * 8     # 192 (x-pad 8: 16B-aligned rows)

TAPS = [(e, f) for e in range(6) for f in range(6)]


def _fold_w(w, g, b, m, v):
    inv = g / np.sqrt(v + EPS)
    Wp = np.zeros((w.shape[0], w.shape[1], 6, 6), np.float32)
    for r in (0, 1):
        for s_ in (0, 1):
            Wp[:, :, r:r + 5, s_:s_ + 5] += w
    Wp *= 0.25 * inv[:, None, None, None]
    beta = (b - m * inv).astype(np.float32)
    return Wp.astype(np.float32), beta


def _lane_rep(a, groups=4):
    out = np.zeros((128, a.shape[1]), np.float32)
    for j in range(groups):
        out[32 * j:32 * j + a.shape[0]] = a
    return out


def host_prep_weights(inputs):
    W1, beta1 = _fold_w(inputs['w1'], inputs['g1'], inputs['b1'], inputs['m1'], inputs['v1'])
    W2, beta2 = _fold_w(inputs['w2'], inputs['g2'], inputs['b2'], inputs['m2'], inputs['v2'])
    W3, beta3 = _fold_w(inputs['w3'], inputs['g3'], inputs['b3'], inputs['m3'], inputs['v3'])
    inv4 = inputs['g4'] / np.sqrt(inputs['v4'] + EPS)
    beta4 = (inputs['b4'] - inputs['m4'] * inv4).astype(np.float32)
    W4 = (inputs['w4'] * inv4[:, None, None, None]).astype(np.float32)

    d = {}
    t1 = W1.reshape(32, 3, 6, 3, 2)            # co, ci, dy, f2, fb
    w36 = t1.transpose(4, 2, 1, 3, 0).reshape(36, 96)
    wl1 = np.zeros((128, 96), np.float32)
    wl1[0:36] = w36
    wl1[64:100] = w36
    d['wl1'] = wl1.astype(NP8)
    d['wl2'] = _lane_rep(W2.transpose(1, 2, 3, 0).reshape(32, 36 * 32)).astype(np.float16)
    d['wl3'] = _lane_rep(W3.transpose(1, 2, 3, 0).reshape(32, 36 * 64)).astype(np.float16)
    wl4 = W4.transpose(1, 2, 3, 0).reshape(64, 16 * 10)
    out4 = np.zeros((128, 160), np.float32)
    out4[0:64] = wl4
    out4[64:128] = wl4
    d['wl4'] = out4.astype(np.float16)

    bt = np.zeros((128, 4), np.float32)
    bt[:, 0] = np.tile(beta1, 4)
    bt[:, 1] = np.tile(beta2, 4)
    bt[:, 2] = np.tile(beta3, 2)
    bt[0:10, 3] = beta4
    d['betas'] = bt
    return d


def host_prep_x(x_core):
    """[128, 3, 32, 32] -> x_l1 [2, 36, 64, 288] column-parity im2row (fp8)."""
    Sc = x_core.shape[0]
    xp = np.zeros((Sc, 3, 36, 36), np.float32)
    xp[:, :, 2:34, 2:34] = x_core
    a = np.stack([xp[:, :, dy:dy + 32:2, :] for dy in range(6)], axis=1)
    b = np.stack([a[..., fb::2] for fb in range(2)], axis=1)
    c = b.reshape(Sc, 36, 288)
    x_l1 = c.reshape(2, 64, 36, 288).transpose(0, 2, 1, 3)
    return np.ascontiguousarray(x_l1).astype(NP8)


def build_program():
    nc = bacc.Bacc(target_bir_lowering=False)

    x_l1 = nc.dram_tensor("x_l1", [2, 36, 64, 288], DT8, kind="ExternalInput")
    wl1 = nc.dram_tensor("wl1", [128, 96], DT8, kind="ExternalInput")
    wl2 = nc.dram_tensor("wl2", [128, 1152], DT, kind="ExternalInput")
    wl3 = nc.dram_tensor("wl3", [128, 2304], DT, kind="ExternalInput")
    wl4 = nc.dram_tensor("wl4", [128, 160], DT, kind="ExternalInput")
    betas = nc.dram_tensor("betas", [128, 4], F32, kind="ExternalInput")
    y = nc.dram_tensor("y", [10, 128], F32, kind="ExternalOutput")

    with TileContext(nc) as tc:
        with tc.tile_pool(name="consts", bufs=1) as cpool, \
             tc.tile_pool(name="ps", bufs=4, space="PSUM") as pspool:
            wdum = cpool.tile([128, 32], DT, name="wdum")
            rdum = cpool.tile([128, 512], DT, name="rdum")
            nc.gpsimd.memset(wdum[:, :], 0.03)
            nc.gpsimd.memset(rdum[:, :], 0.02)

            wl1_t = cpool.tile([128, 96], DT8, name="wl1_t")
            wl2_t = cpool.tile([128, 1152], DT, name="wl2_t")
            wl3_t = cpool.tile([128, 2304], DT, name="wl3_t")
            wl4_t = cpool.tile([128, 160], DT, name="wl4_t")
            betas_t = cpool.tile([128, 4], F32, name="betas_t")
            nc.sync.dma_start(wl1_t[:, :], wl1.ap())
            nc.sync.dma_start(betas_t[:, :], betas.ap())

            # activation tensors (column-parity plane layouts)
            l2in = [cpool.tile([128, Q * L2SLOT], DT, name=f"l2in{i}") for i in range(2)]
            l3in = cpool.tile([128, 2 * Q * L3SLOT], DT, name="l3in")
            stag = cpool.tile([128, 2 * NQ * 16], DT, name="stag")
            out_sb = cpool.tile([128, 128], F32, name="out_sb")

            # input chunks: one tile per 2 sets, first pair split per set
            chunks = [cpool.tile([128, 16 * 288], DT8, name=f"ch{i}") for i in range(4)]
            _issued = set()

            def issue_pair(i, split=False):
                if i in _issued:
                    return
                _issued.add(i)
                chd = chunks[i].rearrange("p (j v) -> p j v", v=288)
                for L in range(2):
                    eng = [nc.sync, nc.scalar][L]
                    if split:
                        for h in range(2):
                            eng.dma_start(
                                chd[64 * L:64 * L + 36, 8 * h:8 * h + 8, :],
                                x_l1.ap()[L, :, 16 * i + 8 * h:16 * i + 8 * h + 8, :])
                    else:
                        eng.dma_start(chd[64 * L:64 * L + 36, 0:16, :],
                                      x_l1.ap()[L, :, 16 * i:16 * i + 16, :])

            issue_pair(0, split=True)
            issue_pair(1)

            # ---- zero the window tensors once (borders stay zero; the
            # evacs overwrite interiors and the clips keep zeros zero).
            # tensor_scalar_mul runs in 4x DVE mode; MEMSET only 1x. ----
            for t_ in l2in:
                nc.vector.tensor_scalar_mul(t_[:, :], t_[:, :], 0.0)
            nc.vector.tensor_scalar_mul(l3in[:, :], l3in[:, :], 0.0)

            psd = [pspool.tile([128, 512], F32, name=f"psd{i}", tag="l1") for i in range(2)]
            for i in range(12):
                nc.tensor.matmul(psd[i % 2][0:32, 0:512], wdum[0:128, 0:32],
                                 rdum[0:128, 0:512], start=True, stop=True,
                                 skip_group_check=True, tile_position=(0, 0))

            issue_pair(2)
            issue_pair(3)
            nc.scalar.dma_start(wl2_t[:, :], wl2.ap())
            nc.scalar.dma_start(wl3_t[:, :], wl3.ap())
            nc.scalar.dma_start(wl4_t[:, :], wl4.ap())

            def evac(engine_is_act, dst, src_ap, bias_ap):
                if engine_is_act:
                    nc.scalar.activation(dst, src_ap, AF.Relu,
                                         bias=bias_ap, scale=1.0)
                else:
                    nc.vector.tensor_scalar(
                        dst, src_ap, bias_ap, 0.0,
                        mybir.AluOpType.add, mybir.AluOpType.max)

            def dummy_burst(nm, n):
                pd = pspool.tile([128, 512], F32, name=nm, tag="l1")
                for _ in range(n):
                    nc.tensor.matmul(pd[0:32, 0:512], wdum[0:128, 0:32],
                                     rdum[0:128, 0:512], start=True, stop=True,
                                     skip_group_check=True, tile_position=(0, 0))

            # ================= L1 (one 16-sample set) =================
            l2vv = [t_.rearrange("p (s fb y x) -> p s fb y x", s=Q, fb=2, y=20)
                    for t_ in l2in]

            def emit_L1_set(t):
                w, u = t // 4, t % 4
                chv = chunks[t // 2].rearrange("p (j y x) -> p j y x", j=16, y=16)
                j0 = 8 * (t % 2)
                ps = [pspool.tile([128, 512], F32, name=f"ps1_{t}_{L}", tag="l1")
                      for L in range(2)]
                for f2 in range(3):
                    for L in range(2):
                        lhsT = wl1_t[64 * L:64 * L + 36, 32 * f2:32 * f2 + 32]
                        for C in range(4):
                            rhs = chv[64 * L:64 * L + 36, j0 + 2 * C:j0 + 2 * C + 2,
                                      0:16, f2:f2 + 16]
                            nc.tensor.matmul(
                                ps[L][32 * C:32 * C + 32, 0:512], lhsT, rhs,
                                start=(f2 == 0), stop=(f2 == 2),
                                skip_group_check=True,
                                tile_position=(64 * L, 32 * C))
                # evac into parity planes: out col 2+X -> plane X%2, x' = 1 + X//2
                vv = l2vv[w % 2]
                for L in range(2):
                    m0 = 4 * u + 2 * L
                    srcv = ps[L][0:128, :].rearrange(
                        "p (k yy xx) -> p k yy xx", k=2, yy=16)
                    for fb in range(2):
                        src_ap = srcv[:, :, :, fb:16:2]
                        dst = vv[0:128, m0:m0 + 2, fb, 2:18, 1:9]
                        eng_act = (L == 0) or (t % 2 == 1 and fb == 0)
                        evac(eng_act, dst, src_ap, betas_t[0:128, 0:1])
                if t % 2 == 1:
                    base = 4 * (u - 1) * L2SLOT
                    nc.vector.tensor_scalar_min(
                        l2in[w % 2][:, base:base + 8 * L2SLOT],
                        l2in[w % 2][:, base:base + 8 * L2SLOT], 1.0)

            # ================= L2 (one wave) =================
            l3vv = l3in.rearrange("p (s fb y x) -> p s fb y x", s=2 * Q, fb=2, y=12)

            def emit_L2(w, tag):
                l2v = l2vv[w % 2]
                pl2 = [pspool.tile([128, 512], F32, name=f"ps2_{w}_{r}", tag=tag)
                       for r in range(4)]
                for t, (e, f) in enumerate(TAPS):
                    fb, f2 = f % 2, f // 2
                    for r2 in range(4):
                        lhsT = wl2_t[32 * r2:32 * r2 + 32, 32 * t:32 * t + 32]
                        for c2 in range(2):
                            rhs = l2v[32 * r2:32 * r2 + 32,
                                      H2 * c2:H2 * (c2 + 1), fb,
                                      e:e + 15:2, f2:f2 + 8]
                            nc.tensor.matmul(
                                pl2[r2][32 * c2:32 * c2 + 32, 0:H2 * 64], lhsT, rhs,
                                start=(t == 0), stop=(t == 35),
                                skip_group_check=True,
                                tile_position=(32 * r2, 32 * c2))
                for r2 in range(4):
                    gb = 64 * (r2 % 2)
                    slot0 = w * Q + H2 * (r2 // 2)
                    srcv = pl2[r2][0:64, 0:H2 * 64].rearrange(
                        "p (k yy xx) -> p k yy xx", k=H2, yy=8)
                    for fb in range(2):
                        src_ap = srcv[:, :, :, fb:8:2]
                        dst = l3vv[gb:gb + 64, slot0:slot0 + H2, fb, 2:10, 1:5]
                        evac(r2 < 2, dst, src_ap, betas_t[gb:gb + 64, 1:2])
                nc.vector.tensor_scalar_min(
                    l3in[:, w * Q * L3SLOT:(w + 1) * Q * L3SLOT],
                    l3in[:, w * Q * L3SLOT:(w + 1) * Q * L3SLOT], 1.0)

            # ================= L3 (both waves) =================
            sv = stag.rearrange("p (g n t) -> p g n t", g=2, n=NQ)

            def emit_L3(tag):
                pl3 = [pspool.tile([128, 512], F32, name=f"ps3_{r}", tag=tag)
                       for r in range(4)]
                for t, (e, f) in enumerate(TAPS):
                    fb, f2 = f % 2, f // 2
                    for r3 in range(4):
                        rhs = l3vv[32 * r3:32 * r3 + 32, :, fb, e:e + 7:2, f2:f2 + 4]
                        for h in range(2):
                            lhsT = wl3_t[32 * r3:32 * r3 + 32,
                                         64 * t + 32 * h:64 * t + 32 * h + 32]
                            nc.tensor.matmul(
                                pl3[r3][32 * h:32 * h + 32, 0:512], lhsT, rhs,
                                start=(t == 0), stop=(t == 35),
                                skip_group_check=True,
                                tile_position=(32 * r3, 32 * h))
                for r3 in range(4):
                    c3, g = r3 // 2, r3 % 2
                    src_ap = pl3[r3][0:64, :].rearrange("p (n t) -> p n t", n=NQ)
                    dst = sv[64 * c3:64 * c3 + 64, g, :, :]
                    evac(r3 < 2, dst, src_ap, betas_t[64 * c3:64 * c3 + 64, 2:3])
                for g in range(2):
                    nc.vector.tensor_scalar_min(
                        stag[:, g * NQ * 16:(g + 1) * NQ * 16],
                        stag[:, g * NQ * 16:(g + 1) * NQ * 16], 1.0)

            # ================= L4 =================
            def emit_L4(tag):
                ps4 = [pspool.tile([128, 64], F32, name=f"ps4_{r}", tag=tag)
                       for r in range(2)]
                for t in range(16):
                    for r in range(2):
                        lhsT = wl4_t[64 * r:64 * r + 64, 10 * t:10 * t + 10]
                        rhs = sv[64 * r:64 * r + 64, :, :, t]
                        nc.tensor.matmul(
                            ps4[r][0:10, 0:64], lhsT, rhs,
                            start=(t == 0), stop=(t == 15),
                            skip_group_check=True,
                            tile_position=(64 * r, 0))
                for r in range(2):
                    nc.scalar.activation(
                        out_sb[0:10, 64 * r:64 * r + 64], ps4[r][0:10, 0:64],
                        AF.Identity, bias=betas_t[0:10, 3:4], scale=1.0)
                    nc.sync.dma_start(y.ap()[:, 64 * r:64 * r + 64],
                                      out_sb[0:10, 64 * r:64 * r + 64])

            # ---- emission order ----
            for t in range(NSET):
                emit_L1_set(t)
                if t < 7:
                    dummy_burst(f"dbs{t}", 3)
            dummy_burst("db0", 3)
            emit_L2(0, "l2")
            emit_L2(1, "l1")
            dummy_burst("db2", 3)
            emit_L3("l2")
            dummy_burst("db3", 5)
            emit_L4("l1")

        return nc


_NC_CACHE = None


def get_program():
    global _NC_CACHE
    if _NC_CACHE is None:
        nc = build_program()
        if not nc.is_finalized():
            nc.finalize()
        _NC_CACHE = nc
    return _NC_CACHE


def make_in_maps(inputs, n_cores=N_CORES):
    wdict = host_prep_weights(inputs)
    in_maps = []
    for c in range(n_cores):
        x_core = np.asarray(inputs['x_in'][c * S:(c + 1) * S], np.float32)
        m = {'x_l1': host_prep_x(x_core)}
        m.update(wdict)
        in_maps.append(m)
    return in_maps


def _col_to_sample(col):
    r, rem = divmod(col, 64)
    g, n = divmod(rem, 32)
    w, slot16 = divmod(n, 16)
    sh, slotrel = divmod(slot16, 8)
    r2 = 2 * sh + r
    C = r2
    m = 8 * g + slotrel
    u, mr = divmod(m, 4)
    L, k = divmod(mr, 2)
    t = 4 * w + u
    j = 8 * t + 2 * C + k
    return 64 * L + j


_COLMAP = np.array([_col_to_sample(c) for c in range(128)])


def assemble_output(results, n_cores=N_CORES):
    out = np.zeros((n_cores * S, 10), np.float32)
    for c in range(n_cores):
        yc = np.asarray(results[c]['y'])
        out[c * S + _COLMAP, :] = yc.T
    return out


def kernel(**inputs) -> np.ndarray:
    from concourse.bass_utils import run_bass_kernel_spmd
    nc = get_program()
    in_maps = make_in_maps(inputs)
    res = run_bass_kernel_spmd(nc, in_maps, list(range(N_CORES)))
    return assemble_output(res.results)
